# revision 83
# baseline (speedup 1.0000x reference)
"""Trainium2 Bass kernel v3 for nn_Aggregator (Linear -> LayerNorm -> segment mean).

Design (v3, ~226 us HW vs 326-383 us baseline):
  - Householder rotation on host: R maps bpp=(b-mean(b)) to |b|*ones/sqrt(D),
    W~ = R@(W-mean).  Device computes q~ = x @ W~^T (NO bias matmuls); the
    bias is a single CONSTANT |b|/sqrt(D) added to every element, applied for
    free by the ACT evac's bias slot (and by ACT Square's pre-affine for the
    squares that run on ACT straight from PSUM).  Output un-rotated on host
    (out @ R), where invcnt is also applied.
  - ssq via square (split ~1:8 ACT:DVE, pattern-tuned) + bf16 fold tree
    (128->64->32->16->8) + 8-wide tensor_reduce: tensor_reduce is 1x-only on
    DVE while TT folds run 2x, and DVE drain scales with output size, so
    fold-then-narrow-reduce wins.
  - STAIRCASE one-hot (key win): tokens are sorted, so each 128-token tile
    touches only <=9 consecutive segments of its 64-seg window.  Host ships a
    CN(=12)-column one-hot per tile plus a per-tile-slot column offset baked
    into the program (union span over all 8 cores; window token counts are
    within ~2% of each other so offsets are core-consistent).  seg-mm N drops
    64->12, sel work drops 5x, one-hot DMA drops 8.2MB->1.5MB.  psum_seg is
    zero-initialized by a K=1 zero-matmul, then staircase slices accumulate
    with start=False + skip_group_check.
  - seg-mm orientation: lhsT = h4 tile [tok,128feat] (stationary), rhs = sel
    tile [tok,CN] (moving) -> psum [feat, seg]; output transposed, host fixes.
  - sel = onehot(fp8) * rstd-broadcast on GPSIMD (broadcast operands force
    DVE to 1x anyway; GPSIMD is otherwise idle).
  - drains emitted one window late (DVE would head-of-line block on seg-mms),
    output streamed to DRAM every 8 windows, window-granular ~1MB x DMAs.

All bf16 on the math path: fp8 for x/h4/sel measured at 2.2e-2..3.6e-2 rel
err vs the 2e-2 gate (rejected); this path measures ~3.0e-3.
"""

import numpy as np

P = 128
D = 128
NSEG = 16384
NCORES = 8
SEG_PER_CORE = NSEG // NCORES    # 2048
WSEG = 64                        # segments per window
NWIN = SEG_PER_CORE // WSEG      # 32 windows per core
EPS = 1e-5
GMAX = 8                         # tiles per PSUM chunk (2 banks)
SQ_PAT = "ADDDDDDD"              # square engine per chunk: A=ACT(psum) D=DVE
SEL_ON_GPS = True                # sel = onehot*rstd on GPSIMD instead of DVE


def _build_program(tw_list, c0_list, CN, nbD):
    import concourse.tile as tile
    from concourse import bacc, mybir

    f32 = mybir.dt.float32
    bf16 = mybir.dt.bfloat16
    fp8 = mybir.dt.float8e4
    AF = mybir.ActivationFunctionType
    OP = mybir.AluOpType

    nwin = len(tw_list)
    TWMAX = max(tw_list)
    NTILES = sum(tw_list)
    NTOK = NTILES * P

    nc = bacc.Bacc(None, target_bir_lowering=False)
    xt = nc.dram_tensor("xt", [P, NTOK], bf16, kind="ExternalInput")
    btb = nc.dram_tensor("btb", [P, NTILES * CN], fp8, kind="ExternalInput")
    # f32 consts: invcnt replicated [128, nwin*64]
    cstf = nc.dram_tensor("cstf", [P, nwin * WSEG], f32, kind="ExternalInput")
    # bf16 consts: wa [128,128] = W~^T
    cstb = nc.dram_tensor("cstb", [P, D], bf16, kind="ExternalInput")
    outd = nc.dram_tensor("out", [P, nwin * WSEG], f32, kind="ExternalOutput")

    with tile.TileContext(nc) as tc:
        with (
            tc.tile_pool(name="const", bufs=1) as cpool,
            tc.tile_pool(name="xw", bufs=4) as xpool,
            tc.tile_pool(name="h4", bufs=10) as hpool,
            tc.tile_pool(name="sq", bufs=3) as sqpool,
            tc.tile_pool(name="c1", bufs=2) as c1pool,
            tc.tile_pool(name="c2", bufs=2) as c2pool,
            tc.tile_pool(name="c3", bufs=2) as c3pool,
            tc.tile_pool(name="c4", bufs=2) as c4pool,
            tc.tile_pool(name="wst", bufs=8) as wpool,
            tc.tile_pool(name="selp", bufs=3) as selpool,
            tc.tile_pool(name="ph", bufs=3, space="PSUM") as phpool,
            tc.tile_pool(name="ps", bufs=2, space="PSUM") as pspool,
        ):
            cf_sb = cpool.tile([P, nwin * WSEG], f32, tag="cstf")
            nc.sync.dma_start(cf_sb[:], cstf[:])
            wa_sb = cpool.tile([P, D], bf16, tag="cstb")
            nc.sync.dma_start(wa_sb[:], cstb[:])
            btc = cpool.tile([P, NTILES * CN], fp8, tag="btc")
            nc.sync.dma_start(btc[:], btb[:])
            zc = cpool.tile([1, D], bf16, tag="zc")
            nc.gpsimd.memset(zc[:], 0.0)
            sbias = cpool.tile([P, 1], f32, tag="sbias")
            nc.gpsimd.memset(sbias[:], float(EPS))
            nbias = cpool.tile([P, 1], f32, tag="nbias")
            nc.gpsimd.memset(nbias[:], float(nbD))
            outb = cpool.tile([P, nwin * WSEG], f32, tag="outb")

            state = {}   # per-window live tiles
            ckidx = 0    # global chunk counter (for SQ_PAT)

            def emit_h(w):
                nonlocal ckidx
                TW = tw_list[w]
                jbase = sum(tw_list[:w])
                xw = xpool.tile([P, TWMAX * D], bf16, tag="xw", name=f"xw{w}")
                nc.sync.dma_start(
                    xw[:, 0: TW * D], xt[:, jbase * P: (jbase + TW) * P])
                ssq_w = wpool.tile([P, TWMAX], f32, tag="ssq", name=f"ssq{w}")
                sq = sqpool.tile([P, TWMAX * D], bf16, tag="sq",
                                 name=f"sq{w}")
                h4s = []
                ngroups = (TW + GMAX - 1) // GMAX
                for g in range(ngroups):
                    g0 = g * GMAX
                    gn = min(GMAX, TW - g0)
                    gc = gn * D
                    psum_h = phpool.tile([P, GMAX * D], f32, tag="ph",
                                         name=f"ph{w}_{g}")
                    for t in range(gn):
                        nc.tensor.matmul(
                            psum_h[:, t * D: (t + 1) * D],
                            xw[:, (g0 + t) * D: (g0 + t + 1) * D], wa_sb,
                            start=True, stop=True,
                        )
                    h4 = hpool.tile([P, GMAX * D], bf16, tag="h4",
                                    name=f"h4_{w}_{g}")
                    # evac + bias in one: h~ = psum + |b|/sqrt(D)  (bias is
                    # constant over ALL features thanks to the ones-rotation)
                    nc.scalar.activation(h4[:, 0:gc], psum_h[:, 0:gc],
                                         AF.Copy, bias=float(nbD))
                    h4s.append((h4, gn))
                    sqs = sq[:, g0 * D: g0 * D + gc]
                    if SQ_PAT[ckidx % len(SQ_PAT)] == "A":
                        # square straight from PSUM; ACT's pre-affine adds
                        # the bias before squaring: (q~ + b)^2
                        nc.scalar.activation(sqs, psum_h[:, 0:gc],
                                             AF.Square, bias=nbias[:])
                    else:
                        nc.vector.tensor_tensor(
                            sqs, h4[:, 0:gc], h4[:, 0:gc], op=OP.mult)
                    ckidx += 1
                # window-granular fold tree: 128 -> 64 -> 32 -> 16 -> reduce
                sqv = sq[:, 0:TW * D].rearrange("p (g n) -> p g n", n=D)
                c1 = c1pool.tile([P, TWMAX * 64], bf16, tag="c1",
                                 name=f"c1_{w}")
                c1v = c1[:, 0:TW * 64].rearrange("p (g n) -> p g n", n=64)
                nc.vector.tensor_tensor(
                    c1v, sqv[:, :, 0:64], sqv[:, :, 64:128], op=OP.add)
                c2 = c2pool.tile([P, TWMAX * 32], bf16, tag="c2",
                                 name=f"c2_{w}")
                c2v = c2[:, 0:TW * 32].rearrange("p (g n) -> p g n", n=32)
                nc.vector.tensor_tensor(
                    c2v, c1v[:, :, 0:32], c1v[:, :, 32:64], op=OP.add)
                c3 = c3pool.tile([P, TWMAX * 16], bf16, tag="c3",
                                 name=f"c3_{w}")
                c3v = c3[:, 0:TW * 16].rearrange("p (g n) -> p g n", n=16)
                nc.vector.tensor_tensor(
                    c3v, c2v[:, :, 0:16], c2v[:, :, 16:32], op=OP.add)
                c4 = c4pool.tile([P, TWMAX * 8], bf16, tag="c4",
                                 name=f"c4_{w}")
                c4v = c4[:, 0:TW * 8].rearrange("p (g n) -> p g n", n=8)
                nc.vector.tensor_tensor(
                    c4v, c3v[:, :, 0:8], c3v[:, :, 8:16], op=OP.add)
                nc.vector.tensor_reduce(
                    ssq_w[:, 0:TW], c4v,
                    axis=mybir.AxisListType.X, op=OP.add,
                )
                state[w] = (h4s, ssq_w, jbase, TW)

            pending_drain = {}

            def emit_seg(w):
                # drain of the PREVIOUS window first: emitted one window late
                # so its deps (seg-mms) are long done when DVE dequeues it --
                # otherwise DVE head-of-line blocks ~6us per window.
                if w - 1 in pending_drain:
                    ps_prev = pending_drain.pop(w - 1)
                    nc.vector.tensor_tensor(
                        outb[:, (w - 1) * WSEG: w * WSEG],
                        ps_prev[:],
                        cf_sb[:, (w - 1) * WSEG: w * WSEG],
                        op=OP.mult)
                    if w % 8 == 0:
                        nc.sync.dma_start(
                            outd[:, (w - 8) * WSEG: w * WSEG],
                            outb[:, (w - 8) * WSEG: w * WSEG])
                h4s, ssq_w, jbase, TW = state.pop(w)
                s_w = wpool.tile([P, TWMAX], f32, tag="sw", name=f"sw{w}")
                nc.scalar.activation(s_w[:, 0:TW], ssq_w[:, 0:TW], AF.Sqrt,
                                     scale=1.0 / D, bias=sbias[:])
                rstd = wpool.tile([P, TWMAX], f32, tag="rstd", name=f"rstd{w}")
                nc.vector.reciprocal(rstd[:, 0:TW], s_w[:, 0:TW])
                sel = selpool.tile([P, TWMAX * CN], bf16, tag="sel",
                                   name=f"sel{w}")
                eng = nc.gpsimd if SEL_ON_GPS else nc.vector
                eng.tensor_tensor(
                    sel[:, 0: TW * CN].rearrange(
                        "p (t c) -> p t c", c=CN),
                    btc[:, jbase * CN: (jbase + TW) * CN].rearrange(
                        "p (t c) -> p t c", c=CN),
                    rstd[:, 0:TW, None].broadcast_to([P, TW, CN]),
                    op=OP.mult)
                psum_seg = pspool.tile([P, WSEG], f32, tag="pseg",
                                       name=f"pseg{w}")
                # zero-init all 64 columns, then accumulate staircase slices
                nc.tensor.matmul(
                    psum_seg[:], zc[0:1, 0:D], zc[0:1, 0:WSEG],
                    start=True, stop=False, skip_group_check=True)
                t = 0
                for (h4, gn) in h4s:
                    for tt in range(gn):
                        c0 = c0_list[jbase + t]
                        nc.tensor.matmul(
                            psum_seg[:, c0: c0 + CN],
                            h4[:, tt * D: (tt + 1) * D],
                            sel[:, t * CN: (t + 1) * CN],
                            start=False, stop=(t == TW - 1),
                            skip_group_check=True,
                        )
                        t += 1
                pending_drain[w] = psum_seg

            for w in range(nwin):
                emit_h(w)
                if w > 0:
                    emit_seg(w - 1)
            emit_seg(nwin - 1)
            ps_last = pending_drain.pop(nwin - 1)
            nc.vector.tensor_tensor(
                outb[:, (nwin - 1) * WSEG: nwin * WSEG],
                ps_last[:],
                cf_sb[:, (nwin - 1) * WSEG: nwin * WSEG],
                op=OP.mult)
            nc.sync.dma_start(outd[:, 24 * WSEG:], outb[:, 24 * WSEG:])
    return nc


TRACE = False
TRACE_DIR = None
LAST = None


def _prepare(x, batch, W, b, ln_w, ln_b):
    import ml_dtypes
    bf16 = ml_dtypes.bfloat16

    x = np.asarray(x, dtype=np.float32)
    batch = np.asarray(batch).astype(np.int64)
    W = np.asarray(W, dtype=np.float64)
    b = np.asarray(b, dtype=np.float64)
    ln_w = np.asarray(ln_w, dtype=np.float32)
    ln_b = np.asarray(ln_b, dtype=np.float32)
    assert np.all(ln_w == 1.0) and np.all(ln_b == 0.0), \
        "general ln affine not wired"

    # fold LN mean subtraction into weights, then Householder-rotate so the
    # bias is |b| * ones/sqrt(D) in the rotated frame (a constant per elem).
    Wpp = W - W.mean(axis=0, keepdims=True)
    bpp = b - b.mean()
    nb = float(np.linalg.norm(bpp))
    u = np.ones(D) / np.sqrt(D)
    v = bpp - nb * u
    if (v @ v) > 1e-24:
        R = np.eye(D) - 2.0 * np.outer(v, v) / (v @ v)   # R @ bpp = nb * u
    else:
        R = np.eye(D)   # bpp already along u (or zero)
    Wt = R @ Wpp                                          # q~ = x @ Wt^T
    nbD = nb / np.sqrt(D)

    edges = np.searchsorted(batch, np.arange(0, NSEG + 1, WSEG))
    wcounts = np.diff(edges).reshape(NCORES, NWIN)
    tw = np.ceil(wcounts / P).astype(np.int64)
    tw_list = np.maximum(tw.max(axis=0), 1).astype(np.int64)
    NTILES = int(tw_list.sum())
    NTOK = NTILES * P

    # per tile-slot staircase window [c0, c0+CN) covering all cores' spans
    lo = np.full(NTILES, WSEG, np.int64)
    hi = np.full(NTILES, -1, np.int64)
    for c in range(NCORES):
        jt = 0
        for w in range(NWIN):
            g = c * NWIN + w
            s, e = int(edges[g]), int(edges[g + 1])
            seg = (batch[s:e] - g * WSEG).astype(np.int64)
            n = e - s
            for t0 in range(0, n, P):
                j = jt + t0 // P
                tile = seg[t0: min(t0 + P, n)]
                lo[j] = min(lo[j], int(tile.min()))
                hi[j] = max(hi[j], int(tile.max()))
            jt += int(tw_list[w])
    span = np.where(hi >= 0, hi - lo + 1, 1)
    CN = max(8, int(-(-int(span.max()) // 4) * 4))
    c0_list = np.clip(np.where(hi >= 0, lo, 0), 0, WSEG - CN).astype(np.int64)
    assert np.all(np.where(hi >= 0, hi, 0) < c0_list + CN)

    xb = x.astype(bf16)
    in_maps = []
    for c in range(NCORES):
        xt_np = np.zeros((P, NTOK), bf16)
        btb2 = np.zeros((P, NTILES, CN), np.float32)
        iota_cn = np.arange(CN, dtype=np.int64)[None, :]
        col0 = 0
        jt = 0
        for w in range(NWIN):
            g = c * NWIN + w
            s, e = int(edges[g]), int(edges[g + 1])
            n = e - s
            if n:
                xt_np[:, col0: col0 + n] = xb[s:e].T
                btl = (batch[s:e] - g * WSEG).astype(np.int64)
                for t0 in range(0, n, P):
                    tn = min(P, n - t0)
                    j = jt + t0 // P
                    btb2[:tn, j, :] = (
                        (btl[t0: t0 + tn, None] - c0_list[j]) == iota_cn
                    ).astype(np.float32)
            col0 += int(tw_list[w]) * P
            jt += int(tw_list[w])
        assert jt == NTILES
        cnts = np.zeros((P, NWIN, WSEG), np.float32)
        for w in range(NWIN):
            g = c * NWIN + w
            s, e = int(edges[g]), int(edges[g + 1])
            cw = np.bincount((batch[s:e] - g * WSEG).astype(np.int64),
                             minlength=WSEG).astype(np.float32)
            cnts[:, w, :] = (1.0 / np.maximum(cw, 1.0))[None, :]
        in_maps.append({
            "xt": xt_np,
            "btb": btb2.reshape(P, NTILES * CN).astype(
                ml_dtypes.float8_e4m3),
            "cstf": cnts.reshape(P, NWIN * WSEG),
            "cstb": Wt.T.astype(bf16),
        })
    return (in_maps, [int(v) for v in tw_list],
            [int(v) for v in c0_list], CN, nbD, R)


def kernel(x, batch, W, b, ln_w, ln_b):
    from concourse.bass_utils import run_bass_kernel_spmd

    (in_maps, tw_list, c0_list, CN, nbD, R) = _prepare(
        x, batch, W, b, ln_w, ln_b)
    nc = _build_program(tw_list, c0_list, CN, nbD)
    nc.finalize()
    kw = {}
    if TRACE:
        kw = dict(trace=True, tmpdir=TRACE_DIR)
    res = run_bass_kernel_spmd(nc, in_maps, list(range(NCORES)), **kw)
    global LAST
    LAST = res
    # per-core out: [128 feat, 2048 seg] (rotated frame) -> gather, transpose,
    # un-rotate.
    outR = np.concatenate(
        [res.results[c]["out"] for c in range(NCORES)], axis=1
    ).astype(np.float64)                      # [128, 16384]
    out = (outR.T @ R).astype(np.float32)     # [16384, 128]
    return out


# revision 84
# speedup vs baseline: 1.0191x; 1.0191x over previous
"""Trainium2 Bass kernel v3 for nn_Aggregator (Linear -> LayerNorm -> segment mean).

Design (v3, ~226 us HW vs 326-383 us baseline):
  - Householder rotation on host: R maps bpp=(b-mean(b)) to |b|*ones/sqrt(D),
    W~ = R@(W-mean).  Device computes q~ = x @ W~^T (NO bias matmuls); the
    bias is a single CONSTANT |b|/sqrt(D) added to every element, applied for
    free by the ACT evac's bias slot (and by ACT Square's pre-affine for the
    squares that run on ACT straight from PSUM).  Output un-rotated on host
    (out @ R), where invcnt is also applied.
  - ssq via square (split ~1:8 ACT:DVE, pattern-tuned) + bf16 fold tree
    (128->64->32->16->8) + 8-wide tensor_reduce: tensor_reduce is 1x-only on
    DVE while TT folds run 2x, and DVE drain scales with output size, so
    fold-then-narrow-reduce wins.
  - STAIRCASE one-hot (key win): tokens are sorted, so each 128-token tile
    touches only <=9 consecutive segments of its 64-seg window.  Host ships a
    CN(=12)-column one-hot per tile plus a per-tile-slot column offset baked
    into the program (union span over all 8 cores; window token counts are
    within ~2% of each other so offsets are core-consistent).  seg-mm N drops
    64->12, sel work drops 5x, one-hot DMA drops 8.2MB->1.5MB.  psum_seg is
    zero-initialized by a K=1 zero-matmul, then staircase slices accumulate
    with start=False + skip_group_check.
  - seg-mm orientation: lhsT = h4 tile [tok,128feat] (stationary), rhs = sel
    tile [tok,CN] (moving) -> psum [feat, seg]; output transposed, host fixes.
  - sel = onehot(fp8) * rstd-broadcast on GPSIMD (broadcast operands force
    DVE to 1x anyway; GPSIMD is otherwise idle).
  - drains emitted one window late (DVE would head-of-line block on seg-mms),
    output streamed to DRAM every 8 windows, window-granular ~1MB x DMAs.

All bf16 on the math path: fp8 for x/h4/sel measured at 2.2e-2..3.6e-2 rel
err vs the 2e-2 gate (rejected); this path measures ~3.0e-3.
"""

import numpy as np

P = 128
D = 128
NSEG = 16384
NCORES = 8
SEG_PER_CORE = NSEG // NCORES    # 2048
WSEG = 64                        # segments per window
NWIN = SEG_PER_CORE // WSEG      # 32 windows per core
EPS = 1e-5
GMAX = 12                        # tiles per PSUM chunk (3 banks)
SQ_PAT = "ADDDDDDDDDDDDDDD"   # square engine per chunk: A=ACT(psum) D=DVE
SEL_ON_GPS = True                # sel = onehot*rstd on GPSIMD instead of DVE


def _build_program(tw_list, c0_list, CN, nbD):
    import concourse.tile as tile
    from concourse import bacc, mybir

    f32 = mybir.dt.float32
    bf16 = mybir.dt.bfloat16
    fp8 = mybir.dt.float8e4
    AF = mybir.ActivationFunctionType
    OP = mybir.AluOpType

    nwin = len(tw_list)
    TWMAX = max(tw_list)
    NTILES = sum(tw_list)
    NTOK = NTILES * P

    nc = bacc.Bacc(None, target_bir_lowering=False)
    xt = nc.dram_tensor("xt", [P, NTOK], bf16, kind="ExternalInput")
    btb = nc.dram_tensor("btb", [P, NTILES * CN], fp8, kind="ExternalInput")
    # f32 consts: invcnt replicated [128, nwin*64]
    cstf = nc.dram_tensor("cstf", [P, nwin * WSEG], f32, kind="ExternalInput")
    # bf16 consts: wa [128,128] = W~^T
    cstb = nc.dram_tensor("cstb", [P, D], bf16, kind="ExternalInput")
    outd = nc.dram_tensor("out", [P, nwin * WSEG], f32, kind="ExternalOutput")

    with tile.TileContext(nc) as tc:
        with (
            tc.tile_pool(name="const", bufs=1) as cpool,
            tc.tile_pool(name="xw", bufs=4) as xpool,
            tc.tile_pool(name="h4", bufs=10) as hpool,
            tc.tile_pool(name="sq", bufs=3) as sqpool,
            tc.tile_pool(name="c1", bufs=2) as c1pool,
            tc.tile_pool(name="c2", bufs=2) as c2pool,
            tc.tile_pool(name="c3", bufs=2) as c3pool,
            tc.tile_pool(name="c4", bufs=2) as c4pool,
            tc.tile_pool(name="wst", bufs=8) as wpool,
            tc.tile_pool(name="selp", bufs=3) as selpool,
            tc.tile_pool(name="ph", bufs=2, space="PSUM") as phpool,
            tc.tile_pool(name="ps", bufs=2, space="PSUM") as pspool,
        ):
            cf_sb = cpool.tile([P, nwin * WSEG], f32, tag="cstf")
            nc.sync.dma_start(cf_sb[:], cstf[:])
            wa_sb = cpool.tile([P, D], bf16, tag="cstb")
            nc.sync.dma_start(wa_sb[:], cstb[:])
            btc = cpool.tile([P, NTILES * CN], fp8, tag="btc")
            nc.sync.dma_start(btc[:], btb[:])
            zc = cpool.tile([1, D], bf16, tag="zc")
            nc.gpsimd.memset(zc[:], 0.0)
            sbias = cpool.tile([P, 1], f32, tag="sbias")
            nc.gpsimd.memset(sbias[:], float(EPS))
            nbias = cpool.tile([P, 1], f32, tag="nbias")
            nc.gpsimd.memset(nbias[:], float(nbD))
            outb = cpool.tile([P, nwin * WSEG], f32, tag="outb")

            state = {}   # per-window live tiles
            ckidx = 0    # global chunk counter (for SQ_PAT)

            def emit_h(w):
                nonlocal ckidx
                TW = tw_list[w]
                jbase = sum(tw_list[:w])
                xw = xpool.tile([P, TWMAX * D], bf16, tag="xw", name=f"xw{w}")
                nc.sync.dma_start(
                    xw[:, 0: TW * D], xt[:, jbase * P: (jbase + TW) * P])
                ssq_w = wpool.tile([P, TWMAX], f32, tag="ssq", name=f"ssq{w}")
                sq = sqpool.tile([P, TWMAX * D], bf16, tag="sq",
                                 name=f"sq{w}")
                h4s = []
                ngroups = (TW + GMAX - 1) // GMAX
                for g in range(ngroups):
                    g0 = g * GMAX
                    gn = min(GMAX, TW - g0)
                    gc = gn * D
                    psum_h = phpool.tile([P, GMAX * D], f32, tag="ph",
                                         name=f"ph{w}_{g}")
                    for t in range(gn):
                        nc.tensor.matmul(
                            psum_h[:, t * D: (t + 1) * D],
                            xw[:, (g0 + t) * D: (g0 + t + 1) * D], wa_sb,
                            start=True, stop=True,
                        )
                    h4 = hpool.tile([P, GMAX * D], bf16, tag="h4",
                                    name=f"h4_{w}_{g}")
                    # evac + bias in one: h~ = psum + |b|/sqrt(D)  (bias is
                    # constant over ALL features thanks to the ones-rotation)
                    nc.scalar.activation(h4[:, 0:gc], psum_h[:, 0:gc],
                                         AF.Copy, bias=float(nbD))
                    h4s.append((h4, gn))
                    sqs = sq[:, g0 * D: g0 * D + gc]
                    if SQ_PAT[ckidx % len(SQ_PAT)] == "A":
                        # square straight from PSUM; ACT's pre-affine adds
                        # the bias before squaring: (q~ + b)^2
                        nc.scalar.activation(sqs, psum_h[:, 0:gc],
                                             AF.Square, bias=nbias[:])
                    else:
                        nc.vector.tensor_tensor(
                            sqs, h4[:, 0:gc], h4[:, 0:gc], op=OP.mult)
                    ckidx += 1
                # window-granular fold tree: 128 -> 64 -> 32 -> 16 -> reduce
                sqv = sq[:, 0:TW * D].rearrange("p (g n) -> p g n", n=D)
                c1 = c1pool.tile([P, TWMAX * 64], bf16, tag="c1",
                                 name=f"c1_{w}")
                c1v = c1[:, 0:TW * 64].rearrange("p (g n) -> p g n", n=64)
                nc.vector.tensor_tensor(
                    c1v, sqv[:, :, 0:64], sqv[:, :, 64:128], op=OP.add)
                c2 = c2pool.tile([P, TWMAX * 32], bf16, tag="c2",
                                 name=f"c2_{w}")
                c2v = c2[:, 0:TW * 32].rearrange("p (g n) -> p g n", n=32)
                nc.vector.tensor_tensor(
                    c2v, c1v[:, :, 0:32], c1v[:, :, 32:64], op=OP.add)
                c3 = c3pool.tile([P, TWMAX * 16], bf16, tag="c3",
                                 name=f"c3_{w}")
                c3v = c3[:, 0:TW * 16].rearrange("p (g n) -> p g n", n=16)
                nc.vector.tensor_tensor(
                    c3v, c2v[:, :, 0:16], c2v[:, :, 16:32], op=OP.add)
                c4 = c4pool.tile([P, TWMAX * 8], bf16, tag="c4",
                                 name=f"c4_{w}")
                c4v = c4[:, 0:TW * 8].rearrange("p (g n) -> p g n", n=8)
                nc.vector.tensor_tensor(
                    c4v, c3v[:, :, 0:8], c3v[:, :, 8:16], op=OP.add)
                nc.vector.tensor_reduce(
                    ssq_w[:, 0:TW], c4v,
                    axis=mybir.AxisListType.X, op=OP.add,
                )
                state[w] = (h4s, ssq_w, jbase, TW)

            pending_drain = {}

            def emit_seg(w):
                # drain of the PREVIOUS window first: emitted one window late
                # so its deps (seg-mms) are long done when DVE dequeues it --
                # otherwise DVE head-of-line blocks ~6us per window.
                if w - 1 in pending_drain:
                    ps_prev = pending_drain.pop(w - 1)
                    nc.vector.tensor_tensor(
                        outb[:, (w - 1) * WSEG: w * WSEG],
                        ps_prev[:],
                        cf_sb[:, (w - 1) * WSEG: w * WSEG],
                        op=OP.mult)
                    if w % 8 == 0:
                        nc.sync.dma_start(
                            outd[:, (w - 8) * WSEG: w * WSEG],
                            outb[:, (w - 8) * WSEG: w * WSEG])
                h4s, ssq_w, jbase, TW = state.pop(w)
                s_w = wpool.tile([P, TWMAX], f32, tag="sw", name=f"sw{w}")
                nc.scalar.activation(s_w[:, 0:TW], ssq_w[:, 0:TW], AF.Sqrt,
                                     scale=1.0 / D, bias=sbias[:])
                rstd = wpool.tile([P, TWMAX], f32, tag="rstd", name=f"rstd{w}")
                nc.vector.reciprocal(rstd[:, 0:TW], s_w[:, 0:TW])
                sel = selpool.tile([P, TWMAX * CN], bf16, tag="sel",
                                   name=f"sel{w}")
                eng = nc.gpsimd if SEL_ON_GPS else nc.vector
                eng.tensor_tensor(
                    sel[:, 0: TW * CN].rearrange(
                        "p (t c) -> p t c", c=CN),
                    btc[:, jbase * CN: (jbase + TW) * CN].rearrange(
                        "p (t c) -> p t c", c=CN),
                    rstd[:, 0:TW, None].broadcast_to([P, TW, CN]),
                    op=OP.mult)
                psum_seg = pspool.tile([P, WSEG], f32, tag="pseg",
                                       name=f"pseg{w}")
                # zero-init all 64 columns, then accumulate staircase slices
                nc.tensor.matmul(
                    psum_seg[:], zc[0:1, 0:D], zc[0:1, 0:WSEG],
                    start=True, stop=False, skip_group_check=True)
                t = 0
                for (h4, gn) in h4s:
                    for tt in range(gn):
                        c0 = c0_list[jbase + t]
                        nc.tensor.matmul(
                            psum_seg[:, c0: c0 + CN],
                            h4[:, tt * D: (tt + 1) * D],
                            sel[:, t * CN: (t + 1) * CN],
                            start=False, stop=(t == TW - 1),
                            skip_group_check=True,
                        )
                        t += 1
                pending_drain[w] = psum_seg

            for w in range(nwin):
                emit_h(w)
                if w > 0:
                    emit_seg(w - 1)
            emit_seg(nwin - 1)
            ps_last = pending_drain.pop(nwin - 1)
            nc.vector.tensor_tensor(
                outb[:, (nwin - 1) * WSEG: nwin * WSEG],
                ps_last[:],
                cf_sb[:, (nwin - 1) * WSEG: nwin * WSEG],
                op=OP.mult)
            nc.sync.dma_start(outd[:, 24 * WSEG:], outb[:, 24 * WSEG:])
    return nc


TRACE = False
TRACE_DIR = None
LAST = None


def _prepare(x, batch, W, b, ln_w, ln_b):
    import ml_dtypes
    bf16 = ml_dtypes.bfloat16

    x = np.asarray(x, dtype=np.float32)
    batch = np.asarray(batch).astype(np.int64)
    W = np.asarray(W, dtype=np.float64)
    b = np.asarray(b, dtype=np.float64)
    ln_w = np.asarray(ln_w, dtype=np.float32)
    ln_b = np.asarray(ln_b, dtype=np.float32)
    assert np.all(ln_w == 1.0) and np.all(ln_b == 0.0), \
        "general ln affine not wired"

    # fold LN mean subtraction into weights, then Householder-rotate so the
    # bias is |b| * ones/sqrt(D) in the rotated frame (a constant per elem).
    Wpp = W - W.mean(axis=0, keepdims=True)
    bpp = b - b.mean()
    nb = float(np.linalg.norm(bpp))
    u = np.ones(D) / np.sqrt(D)
    v = bpp - nb * u
    if (v @ v) > 1e-24:
        R = np.eye(D) - 2.0 * np.outer(v, v) / (v @ v)   # R @ bpp = nb * u
    else:
        R = np.eye(D)   # bpp already along u (or zero)
    Wt = R @ Wpp                                          # q~ = x @ Wt^T
    nbD = nb / np.sqrt(D)

    edges = np.searchsorted(batch, np.arange(0, NSEG + 1, WSEG))
    wcounts = np.diff(edges).reshape(NCORES, NWIN)
    tw = np.ceil(wcounts / P).astype(np.int64)
    tw_list = np.maximum(tw.max(axis=0), 1).astype(np.int64)
    NTILES = int(tw_list.sum())
    NTOK = NTILES * P

    # per tile-slot staircase window [c0, c0+CN) covering all cores' spans
    lo = np.full(NTILES, WSEG, np.int64)
    hi = np.full(NTILES, -1, np.int64)
    for c in range(NCORES):
        jt = 0
        for w in range(NWIN):
            g = c * NWIN + w
            s, e = int(edges[g]), int(edges[g + 1])
            seg = (batch[s:e] - g * WSEG).astype(np.int64)
            n = e - s
            for t0 in range(0, n, P):
                j = jt + t0 // P
                tile = seg[t0: min(t0 + P, n)]
                lo[j] = min(lo[j], int(tile.min()))
                hi[j] = max(hi[j], int(tile.max()))
            jt += int(tw_list[w])
    span = np.where(hi >= 0, hi - lo + 1, 1)
    CN = max(8, int(-(-int(span.max()) // 4) * 4))
    c0_list = np.clip(np.where(hi >= 0, lo, 0), 0, WSEG - CN).astype(np.int64)
    assert np.all(np.where(hi >= 0, hi, 0) < c0_list + CN)

    xb = x.astype(bf16)
    in_maps = []
    for c in range(NCORES):
        xt_np = np.zeros((P, NTOK), bf16)
        btb2 = np.zeros((P, NTILES, CN), np.float32)
        iota_cn = np.arange(CN, dtype=np.int64)[None, :]
        col0 = 0
        jt = 0
        for w in range(NWIN):
            g = c * NWIN + w
            s, e = int(edges[g]), int(edges[g + 1])
            n = e - s
            if n:
                xt_np[:, col0: col0 + n] = xb[s:e].T
                btl = (batch[s:e] - g * WSEG).astype(np.int64)
                for t0 in range(0, n, P):
                    tn = min(P, n - t0)
                    j = jt + t0 // P
                    btb2[:tn, j, :] = (
                        (btl[t0: t0 + tn, None] - c0_list[j]) == iota_cn
                    ).astype(np.float32)
            col0 += int(tw_list[w]) * P
            jt += int(tw_list[w])
        assert jt == NTILES
        cnts = np.zeros((P, NWIN, WSEG), np.float32)
        for w in range(NWIN):
            g = c * NWIN + w
            s, e = int(edges[g]), int(edges[g + 1])
            cw = np.bincount((batch[s:e] - g * WSEG).astype(np.int64),
                             minlength=WSEG).astype(np.float32)
            cnts[:, w, :] = (1.0 / np.maximum(cw, 1.0))[None, :]
        in_maps.append({
            "xt": xt_np,
            "btb": btb2.reshape(P, NTILES * CN).astype(
                ml_dtypes.float8_e4m3),
            "cstf": cnts.reshape(P, NWIN * WSEG),
            "cstb": Wt.T.astype(bf16),
        })
    return (in_maps, [int(v) for v in tw_list],
            [int(v) for v in c0_list], CN, nbD, R)


def kernel(x, batch, W, b, ln_w, ln_b):
    from concourse.bass_utils import run_bass_kernel_spmd

    (in_maps, tw_list, c0_list, CN, nbD, R) = _prepare(
        x, batch, W, b, ln_w, ln_b)
    nc = _build_program(tw_list, c0_list, CN, nbD)
    nc.finalize()
    kw = {}
    if TRACE:
        kw = dict(trace=True, tmpdir=TRACE_DIR)
    res = run_bass_kernel_spmd(nc, in_maps, list(range(NCORES)), **kw)
    global LAST
    LAST = res
    # per-core out: [128 feat, 2048 seg] (rotated frame) -> gather, transpose,
    # un-rotate.
    outR = np.concatenate(
        [res.results[c]["out"] for c in range(NCORES)], axis=1
    ).astype(np.float64)                      # [128, 16384]
    out = (outR.T @ R).astype(np.float32)     # [16384, 128]
    return out


# revision 85
# speedup vs baseline: 1.0898x; 1.0694x over previous
"""Trainium2 Bass kernel v3 for nn_Aggregator (Linear -> LayerNorm -> segment mean).

Design (v3, ~226 us HW vs 326-383 us baseline):
  - Householder rotation on host: R maps bpp=(b-mean(b)) to |b|*ones/sqrt(D),
    W~ = R@(W-mean).  Device computes q~ = x @ W~^T (NO bias matmuls); the
    bias is a single CONSTANT |b|/sqrt(D) added to every element, applied for
    free by the ACT evac's bias slot (and by ACT Square's pre-affine for the
    squares that run on ACT straight from PSUM).  Output un-rotated on host
    (out @ R), where invcnt is also applied.
  - ssq via square (split ~1:8 ACT:DVE, pattern-tuned) + bf16 fold tree
    (128->64->32->16->8) + 8-wide tensor_reduce: tensor_reduce is 1x-only on
    DVE while TT folds run 2x, and DVE drain scales with output size, so
    fold-then-narrow-reduce wins.
  - STAIRCASE one-hot (key win): tokens are sorted, so each 128-token tile
    touches only <=9 consecutive segments of its 64-seg window.  Host ships a
    CN(=12)-column one-hot per tile plus a per-tile-slot column offset baked
    into the program (union span over all 8 cores; window token counts are
    within ~2% of each other so offsets are core-consistent).  seg-mm N drops
    64->12, sel work drops 5x, one-hot DMA drops 8.2MB->1.5MB.  psum_seg is
    zero-initialized by a K=1 zero-matmul, then staircase slices accumulate
    with start=False + skip_group_check.
  - seg-mm orientation: lhsT = h4 tile [tok,128feat] (stationary), rhs = sel
    tile [tok,CN] (moving) -> psum [feat, seg]; output transposed, host fixes.
  - sel = onehot(fp8) * rstd-broadcast on GPSIMD (broadcast operands force
    DVE to 1x anyway; GPSIMD is otherwise idle).
  - drains emitted one window late (DVE would head-of-line block on seg-mms),
    output streamed to DRAM every 8 windows, window-granular ~1MB x DMAs.

All bf16 on the math path: fp8 for x/h4/sel measured at 2.2e-2..3.6e-2 rel
err vs the 2e-2 gate (rejected); this path measures ~3.0e-3.
"""

import numpy as np

P = 128
D = 128
NSEG = 16384
NCORES = 8
SEG_PER_CORE = NSEG // NCORES    # 2048
WSEG = 64                        # segments per window
NWIN = SEG_PER_CORE // WSEG      # 32 windows per core
EPS = 1e-5
GMAX = 12                        # tiles per PSUM chunk (3 banks)
SQ_PAT = "ADDDDDDD"              # square engine per chunk: A=ACT(psum) D=DVE
SEL_ON_GPS = True                # sel = onehot*rstd on GPSIMD instead of DVE


def _build_program(tw_list, c0_list, CN, nbD):
    import concourse.tile as tile
    from concourse import bacc, mybir

    f32 = mybir.dt.float32
    bf16 = mybir.dt.bfloat16
    fp8 = mybir.dt.float8e4
    AF = mybir.ActivationFunctionType
    OP = mybir.AluOpType

    nwin = len(tw_list)
    TWMAX = max(tw_list)
    NTILES = sum(tw_list)
    NTOK = NTILES * P

    nc = bacc.Bacc(None, target_bir_lowering=False)
    xt = nc.dram_tensor("xt", [P, NTOK], bf16, kind="ExternalInput")
    btb = nc.dram_tensor("btb", [P, NTILES * CN], fp8, kind="ExternalInput")
    # f32 consts: invcnt replicated [128, nwin*64]
    cstf = nc.dram_tensor("cstf", [P, nwin * WSEG], f32, kind="ExternalInput")
    # bf16 consts: wa [128,128] = W~^T
    cstb = nc.dram_tensor("cstb", [P, D], bf16, kind="ExternalInput")
    outd = nc.dram_tensor("out", [P, nwin * WSEG], f32, kind="ExternalOutput")

    with tile.TileContext(nc) as tc:
        with (
            tc.tile_pool(name="const", bufs=1) as cpool,
            tc.tile_pool(name="xw", bufs=4) as xpool,
            tc.tile_pool(name="h4", bufs=10) as hpool,
            tc.tile_pool(name="sq", bufs=3) as sqpool,
            tc.tile_pool(name="c1", bufs=2) as c1pool,
            tc.tile_pool(name="c2", bufs=2) as c2pool,
            tc.tile_pool(name="c3", bufs=2) as c3pool,
            tc.tile_pool(name="c4", bufs=2) as c4pool,
            tc.tile_pool(name="wst", bufs=8) as wpool,
            tc.tile_pool(name="selp", bufs=4) as selpool,
            tc.tile_pool(name="ph", bufs=2, space="PSUM") as phpool,
            tc.tile_pool(name="ps", bufs=2, space="PSUM") as pspool,
        ):
            cf_sb = cpool.tile([P, nwin * WSEG], f32, tag="cstf")
            nc.sync.dma_start(cf_sb[:], cstf[:])
            wa_sb = cpool.tile([P, D], bf16, tag="cstb")
            nc.sync.dma_start(wa_sb[:], cstb[:])
            btc = cpool.tile([P, NTILES * CN], fp8, tag="btc")
            nc.sync.dma_start(btc[:], btb[:])
            zc = cpool.tile([1, D], bf16, tag="zc")
            nc.gpsimd.memset(zc[:], 0.0)
            sbias = cpool.tile([P, 1], f32, tag="sbias")
            nc.gpsimd.memset(sbias[:], float(EPS))
            nbias = cpool.tile([P, 1], f32, tag="nbias")
            nc.gpsimd.memset(nbias[:], float(nbD))
            outb = cpool.tile([P, nwin * WSEG], f32, tag="outb")

            state = {}   # per-window live tiles
            ckidx = 0    # global chunk counter (for SQ_PAT)

            def emit_h(w):
                nonlocal ckidx
                TW = tw_list[w]
                jbase = sum(tw_list[:w])
                xw = xpool.tile([P, TWMAX * D], bf16, tag="xw", name=f"xw{w}")
                nc.sync.dma_start(
                    xw[:, 0: TW * D], xt[:, jbase * P: (jbase + TW) * P])
                ssq_w = wpool.tile([P, TWMAX], f32, tag="ssq", name=f"ssq{w}")
                sq = sqpool.tile([P, TWMAX * D], bf16, tag="sq",
                                 name=f"sq{w}")
                h4s = []
                ngroups = (TW + GMAX - 1) // GMAX
                for g in range(ngroups):
                    g0 = g * GMAX
                    gn = min(GMAX, TW - g0)
                    gc = gn * D
                    psum_h = phpool.tile([P, GMAX * D], f32, tag="ph",
                                         name=f"ph{w}_{g}")
                    for t in range(gn):
                        nc.tensor.matmul(
                            psum_h[:, t * D: (t + 1) * D],
                            xw[:, (g0 + t) * D: (g0 + t + 1) * D], wa_sb,
                            start=True, stop=True,
                        )
                    h4 = hpool.tile([P, GMAX * D], bf16, tag="h4",
                                    name=f"h4_{w}_{g}")
                    # evac + bias in one: h~ = psum + |b|/sqrt(D)  (bias is
                    # constant over ALL features thanks to the ones-rotation)
                    nc.scalar.activation(h4[:, 0:gc], psum_h[:, 0:gc],
                                         AF.Copy, bias=float(nbD))
                    h4s.append((h4, gn))
                    sqs = sq[:, g0 * D: g0 * D + gc]
                    if SQ_PAT[ckidx % len(SQ_PAT)] == "A":
                        # square straight from PSUM; ACT's pre-affine adds
                        # the bias before squaring: (q~ + b)^2
                        nc.scalar.activation(sqs, psum_h[:, 0:gc],
                                             AF.Square, bias=nbias[:])
                    else:
                        nc.vector.tensor_tensor(
                            sqs, h4[:, 0:gc], h4[:, 0:gc], op=OP.mult)
                    ckidx += 1
                # window-granular fold tree: 128 -> 64 -> 32 -> 16 -> reduce
                sqv = sq[:, 0:TW * D].rearrange("p (g n) -> p g n", n=D)
                c1 = c1pool.tile([P, TWMAX * 64], bf16, tag="c1",
                                 name=f"c1_{w}")
                c1v = c1[:, 0:TW * 64].rearrange("p (g n) -> p g n", n=64)
                nc.vector.tensor_tensor(
                    c1v, sqv[:, :, 0:64], sqv[:, :, 64:128], op=OP.add)
                c2 = c2pool.tile([P, TWMAX * 32], bf16, tag="c2",
                                 name=f"c2_{w}")
                c2v = c2[:, 0:TW * 32].rearrange("p (g n) -> p g n", n=32)
                nc.vector.tensor_tensor(
                    c2v, c1v[:, :, 0:32], c1v[:, :, 32:64], op=OP.add)
                c3 = c3pool.tile([P, TWMAX * 16], bf16, tag="c3",
                                 name=f"c3_{w}")
                c3v = c3[:, 0:TW * 16].rearrange("p (g n) -> p g n", n=16)
                nc.vector.tensor_tensor(
                    c3v, c2v[:, :, 0:16], c2v[:, :, 16:32], op=OP.add)
                c4 = c4pool.tile([P, TWMAX * 8], bf16, tag="c4",
                                 name=f"c4_{w}")
                c4v = c4[:, 0:TW * 8].rearrange("p (g n) -> p g n", n=8)
                nc.vector.tensor_tensor(
                    c4v, c3v[:, :, 0:8], c3v[:, :, 8:16], op=OP.add)
                nc.vector.tensor_reduce(
                    ssq_w[:, 0:TW], c4v,
                    axis=mybir.AxisListType.X, op=OP.add,
                )
                state[w] = (h4s, ssq_w, jbase, TW)

            pending_drain = {}

            def emit_seg(w):
                # drain of the PREVIOUS window first: emitted one window late
                # so its deps (seg-mms) are long done when DVE dequeues it --
                # otherwise DVE head-of-line blocks ~6us per window.
                if w - 1 in pending_drain:
                    ps_prev = pending_drain.pop(w - 1)
                    nc.vector.tensor_tensor(
                        outb[:, (w - 1) * WSEG: w * WSEG],
                        ps_prev[:],
                        cf_sb[:, (w - 1) * WSEG: w * WSEG],
                        op=OP.mult)
                    if w % 8 == 0:
                        nc.sync.dma_start(
                            outd[:, (w - 8) * WSEG: w * WSEG],
                            outb[:, (w - 8) * WSEG: w * WSEG])
                h4s, ssq_w, jbase, TW = state.pop(w)
                s_w = wpool.tile([P, TWMAX], f32, tag="sw", name=f"sw{w}")
                nc.scalar.activation(s_w[:, 0:TW], ssq_w[:, 0:TW], AF.Sqrt,
                                     scale=1.0 / D, bias=sbias[:])
                rstd = wpool.tile([P, TWMAX], f32, tag="rstd", name=f"rstd{w}")
                nc.vector.reciprocal(rstd[:, 0:TW], s_w[:, 0:TW])
                sel = selpool.tile([P, TWMAX * CN], bf16, tag="sel",
                                   name=f"sel{w}")
                eng = nc.gpsimd if SEL_ON_GPS else nc.vector
                eng.tensor_tensor(
                    sel[:, 0: TW * CN].rearrange(
                        "p (t c) -> p t c", c=CN),
                    btc[:, jbase * CN: (jbase + TW) * CN].rearrange(
                        "p (t c) -> p t c", c=CN),
                    rstd[:, 0:TW, None].broadcast_to([P, TW, CN]),
                    op=OP.mult)
                psum_seg = pspool.tile([P, WSEG], f32, tag="pseg",
                                       name=f"pseg{w}")
                # zero-init all 64 columns, then accumulate staircase slices
                nc.tensor.matmul(
                    psum_seg[:], zc[0:1, 0:D], zc[0:1, 0:WSEG],
                    start=True, stop=False, skip_group_check=True)
                t = 0
                for (h4, gn) in h4s:
                    for tt in range(gn):
                        c0 = c0_list[jbase + t]
                        nc.tensor.matmul(
                            psum_seg[:, c0: c0 + CN],
                            h4[:, tt * D: (tt + 1) * D],
                            sel[:, t * CN: (t + 1) * CN],
                            start=False, stop=(t == TW - 1),
                            skip_group_check=True,
                        )
                        t += 1
                pending_drain[w] = psum_seg

            for w in range(nwin):
                emit_h(w)
                if w > 0:
                    emit_seg(w - 1)
            emit_seg(nwin - 1)
            ps_last = pending_drain.pop(nwin - 1)
            nc.vector.tensor_tensor(
                outb[:, (nwin - 1) * WSEG: nwin * WSEG],
                ps_last[:],
                cf_sb[:, (nwin - 1) * WSEG: nwin * WSEG],
                op=OP.mult)
            nc.sync.dma_start(outd[:, 24 * WSEG:], outb[:, 24 * WSEG:])
    return nc


TRACE = False
TRACE_DIR = None
LAST = None


def _prepare(x, batch, W, b, ln_w, ln_b):
    import ml_dtypes
    bf16 = ml_dtypes.bfloat16

    x = np.asarray(x, dtype=np.float32)
    batch = np.asarray(batch).astype(np.int64)
    W = np.asarray(W, dtype=np.float64)
    b = np.asarray(b, dtype=np.float64)
    ln_w = np.asarray(ln_w, dtype=np.float32)
    ln_b = np.asarray(ln_b, dtype=np.float32)
    assert np.all(ln_w == 1.0) and np.all(ln_b == 0.0), \
        "general ln affine not wired"

    # fold LN mean subtraction into weights, then Householder-rotate so the
    # bias is |b| * ones/sqrt(D) in the rotated frame (a constant per elem).
    Wpp = W - W.mean(axis=0, keepdims=True)
    bpp = b - b.mean()
    nb = float(np.linalg.norm(bpp))
    u = np.ones(D) / np.sqrt(D)
    v = bpp - nb * u
    if (v @ v) > 1e-24:
        R = np.eye(D) - 2.0 * np.outer(v, v) / (v @ v)   # R @ bpp = nb * u
    else:
        R = np.eye(D)   # bpp already along u (or zero)
    Wt = R @ Wpp                                          # q~ = x @ Wt^T
    nbD = nb / np.sqrt(D)

    edges = np.searchsorted(batch, np.arange(0, NSEG + 1, WSEG))
    wcounts = np.diff(edges).reshape(NCORES, NWIN)
    tw = np.ceil(wcounts / P).astype(np.int64)
    tw_list = np.maximum(tw.max(axis=0), 1).astype(np.int64)
    NTILES = int(tw_list.sum())
    NTOK = NTILES * P

    # per tile-slot staircase window [c0, c0+CN) covering all cores' spans
    lo = np.full(NTILES, WSEG, np.int64)
    hi = np.full(NTILES, -1, np.int64)
    for c in range(NCORES):
        jt = 0
        for w in range(NWIN):
            g = c * NWIN + w
            s, e = int(edges[g]), int(edges[g + 1])
            seg = (batch[s:e] - g * WSEG).astype(np.int64)
            n = e - s
            for t0 in range(0, n, P):
                j = jt + t0 // P
                tile = seg[t0: min(t0 + P, n)]
                lo[j] = min(lo[j], int(tile.min()))
                hi[j] = max(hi[j], int(tile.max()))
            jt += int(tw_list[w])
    span = np.where(hi >= 0, hi - lo + 1, 1)
    CN = max(8, int(-(-int(span.max()) // 4) * 4))
    c0_list = np.clip(np.where(hi >= 0, lo, 0), 0, WSEG - CN).astype(np.int64)
    assert np.all(np.where(hi >= 0, hi, 0) < c0_list + CN)

    xb = x.astype(bf16)
    in_maps = []
    for c in range(NCORES):
        xt_np = np.zeros((P, NTOK), bf16)
        btb2 = np.zeros((P, NTILES, CN), np.float32)
        iota_cn = np.arange(CN, dtype=np.int64)[None, :]
        col0 = 0
        jt = 0
        for w in range(NWIN):
            g = c * NWIN + w
            s, e = int(edges[g]), int(edges[g + 1])
            n = e - s
            if n:
                xt_np[:, col0: col0 + n] = xb[s:e].T
                btl = (batch[s:e] - g * WSEG).astype(np.int64)
                for t0 in range(0, n, P):
                    tn = min(P, n - t0)
                    j = jt + t0 // P
                    btb2[:tn, j, :] = (
                        (btl[t0: t0 + tn, None] - c0_list[j]) == iota_cn
                    ).astype(np.float32)
            col0 += int(tw_list[w]) * P
            jt += int(tw_list[w])
        assert jt == NTILES
        cnts = np.zeros((P, NWIN, WSEG), np.float32)
        for w in range(NWIN):
            g = c * NWIN + w
            s, e = int(edges[g]), int(edges[g + 1])
            cw = np.bincount((batch[s:e] - g * WSEG).astype(np.int64),
                             minlength=WSEG).astype(np.float32)
            cnts[:, w, :] = (1.0 / np.maximum(cw, 1.0))[None, :]
        in_maps.append({
            "xt": xt_np,
            "btb": btb2.reshape(P, NTILES * CN).astype(
                ml_dtypes.float8_e4m3),
            "cstf": cnts.reshape(P, NWIN * WSEG),
            "cstb": Wt.T.astype(bf16),
        })
    return (in_maps, [int(v) for v in tw_list],
            [int(v) for v in c0_list], CN, nbD, R)


def kernel(x, batch, W, b, ln_w, ln_b):
    from concourse.bass_utils import run_bass_kernel_spmd

    (in_maps, tw_list, c0_list, CN, nbD, R) = _prepare(
        x, batch, W, b, ln_w, ln_b)
    nc = _build_program(tw_list, c0_list, CN, nbD)
    nc.finalize()
    kw = {}
    if TRACE:
        kw = dict(trace=True, tmpdir=TRACE_DIR)
    res = run_bass_kernel_spmd(nc, in_maps, list(range(NCORES)), **kw)
    global LAST
    LAST = res
    # per-core out: [128 feat, 2048 seg] (rotated frame) -> gather, transpose,
    # un-rotate.
    outR = np.concatenate(
        [res.results[c]["out"] for c in range(NCORES)], axis=1
    ).astype(np.float64)                      # [128, 16384]
    out = (outR.T @ R).astype(np.float32)     # [16384, 128]
    return out
